# revision 52
# baseline (speedup 1.0000x reference)
"""Causal self-attention (dense transformer block) for 8 Trainium2 NeuronCores.

Sharding: DP over batch (2) x TP over heads (4 groups of 4 heads) = 8 cores.
Per core: column-parallel QKV projection (4 heads), RoPE, causal
flash-attention (no-max-subtraction softmax with constant bias), row-parallel
output projection producing a partial [oc, t] result; host sums the 4 TP
partials per batch and transposes back.

All matmul storage is bf16 (PSUM accumulation stays fp32): same PE rate as
f32r but half the DMA/SBUF traffic and 2x DVE throughput. Score tiles are
processed in [128,1024] two-bank psum pairs (one exp per two key tiles --
ACT fixed overhead is the attention bottleneck); diagonal tiles shrink to
the causal band. The softmax denominator is accumulated on the DVE and
reduced by a single ones-matmul per query block. x stays fully
SBUF-resident across both head-pairs; y^T is never spilled.

Self-contained: hardcodes shapes, builds/compiles/runs the Bass kernel via
run_bass_kernel_spmd on cores 0-7.
"""

import os
import sys
import types

sys.path.insert(0, "/opt/trn_rl_repo")

import numpy as np
import ml_dtypes

import concourse.bass as bass
import concourse.bass_isa as bass_isa
import concourse.mybir as mybir
import concourse.tile as tile
from concourse import bacc
from concourse.bass_utils import run_bass_kernel_spmd
from concourse.vector_clock import ScopedClock, VectorClock

F32 = mybir.dt.float32
BF16 = mybir.dt.bfloat16
AF = mybir.ActivationFunctionType
ALU = mybir.AluOpType

P = 128
T = 2048
C = 2048
NH = 16          # total heads
HPC = 4          # heads per core
HSIZE = 128
N_CORES = 8
TG = 4           # t-groups of 512
QG = 512
EXP_BIAS = -10.0
SCALE = 1.0 / float(np.sqrt(HSIZE))

_TRACE = os.environ.get("BASS_KERNEL_TRACE", "0") == "1"


def _patch_tile_drain():
    """walrus in this toolchain allows at most one sync-wait per instruction;
    TileContext's tail drain aggregates the whole global clock onto one Drain.
    Split it: one Drain per pending proc, each with a single wait."""
    if getattr(tile.TileContext, "_drain_patched", False):
        return

    def _drain_and_barrier(self, tick_clock, wait_clock):
        nc = self.nc
        gc = tick_clock.global_clock
        n = len(gc)
        for p in range(n):
            if gc[p] > 0:
                vc = VectorClock([gc[p] if i == p else 0 for i in range(n)])
                di = nc.sync.drain()
                wait_clock.add_sem_waits(di.ins, ScopedClock({None: vc}))
        nc.all_engine_barrier()
        popped = nc._tile_sem_poison_stack.pop()
        assert popped is self._sem_poison
        nc.clear_and_free_semaphores(list(self.sems.allocated().values()))
        nc.all_engine_barrier()

    tile.TileContext._drain_and_barrier = _drain_and_barrier
    tile.TileContext._drain_patched = True


def _install_ntff_hook():
    """Wire the axon NTFF profiling hook this image leaves unwired (the agent
    image's antenv lacks axon_hooks). Only needed when tracing."""
    import antenv

    if getattr(antenv, "axon_hooks", None) is not None:
        return
    mod = types.ModuleType("antenv.axon_hooks")
    mod._hook = None
    mod.set_axon_ntff_profile_hook = lambda h: setattr(mod, "_hook", h)
    mod.get_axon_ntff_profile_hook = lambda: mod._hook
    sys.modules["antenv.axon_hooks"] = mod
    antenv.axon_hooks = mod
    if "/root/.axon_site" not in sys.path:
        sys.path.insert(0, "/root/.axon_site")
    try:
        from trn_agent_boot.trn_boot import _ntff_profile_via_ctypes

        hook = _ntff_profile_via_ctypes("/opt/axon/libaxon_pjrt.so")
        if hook is not None:
            mod.set_axon_ntff_profile_hook(hook)
        import concourse.bass_utils as bu

        bu.upload_artifacts = lambda d: d
    except Exception:
        pass


def build_nc():
    _patch_tile_drain()
    nc = bacc.Bacc(None, target_bir_lowering=False)

    xT = nc.dram_tensor("xT", [C, T], BF16, kind="ExternalInput")
    w = nc.dram_tensor("w", [C, 6 * HSIZE * 2], BF16, kind="ExternalInput")  # [C,1536]
    wp = nc.dram_tensor("wp", [HPC * HSIZE, T], BF16, kind="ExternalInput")  # [512,T]
    c1d = nc.dram_tensor("c1", [P, T], BF16, kind="ExternalInput")
    c2d = nc.dram_tensor("c2", [P, T], BF16, kind="ExternalInput")
    mkd = nc.dram_tensor("mk", [2, P, 2 * QG], BF16, kind="ExternalInput")
    onesd = nc.dram_tensor("ones_col", [P, 1], BF16, kind="ExternalInput")
    swpd = nc.dram_tensor("swp", [P, P], BF16, kind="ExternalInput")
    outT = nc.dram_tensor("outT", [T, T], BF16, kind="ExternalOutput")  # [oc, t]

    xTr = xT.rearrange("(cc p) t -> p cc t", p=P)      # [128,16,2048]
    wr = w.rearrange("(cc p) j -> p cc j", p=P)        # [128,16,1536]
    wpr = wp.rearrange("(hc p) t -> p hc t", p=P)      # [128,4,2048]
    mkr = mkd.rearrange("s p q -> p s q")              # [128,4,512]

    def r(ap):
        return ap

    with tile.TileContext(nc) as tc, nc.allow_low_precision(
        reason="bf16 storage matmuls; fp32 PSUM accumulation"
    ):
        with (
            tc.tile_pool(name="const", bufs=1) as constp,
            tc.tile_pool(name="wpool", bufs=1) as wpool,
            tc.tile_pool(name="wppool", bufs=1) as wppool,
            tc.tile_pool(name="xres", bufs=1) as xres,
            tc.tile_pool(name="qk", bufs=1) as qkres,
            tc.tile_pool(name="vres", bufs=1) as vresp,
            tc.tile_pool(name="ytpool", bufs=1) as ytpool,
            tc.tile_pool(name="work", bufs=8) as work,
            tc.tile_pool(name="pwork", bufs=7) as pwork,
            tc.tile_pool(name="lacc", bufs=2) as laccp,
            tc.tile_pool(name="rp", bufs=1) as rpool,
            tc.tile_pool(name="rbig", bufs=2) as rbigp,
            tc.tile_pool(name="mm", bufs=2, space="PSUM") as mmp,
            tc.tile_pool(name="yt", bufs=2, space="PSUM") as ytp,
            tc.tile_pool(name="lp", bufs=2, space="PSUM") as lpp,
        ):
            # ---- PE warmup first: no DMA dependency at all ----
            warm = constp.tile([P, QG], BF16, tag="warm")
            nc.vector.memset(warm[:], 0.0)
            ps_wu = mmp.tile([P, 2 * QG], F32, tag="mm", name="ps_wu")
            for _ in range(12):
                nc.tensor.matmul(
                    ps_wu[:, 0:QG], warm[:, 0:P], warm[:], start=True, stop=True
                )

            # ---- priority DMAs ----
            # separate tiles per 2-cc chunk, DMA'd per column-third (jp0 q
            # cols | jp1 k cols | v cols) interleaved with tg0's x: the DMA
            # semaphore merges waits, so the first jp pass must only sit
            # behind ~1MB of weight traffic, not the full 3.1MB
            # x stays resident all kernel: 16 tiles of [128, 4cc, 512t]
            xtiles = [[None] * 4 for _ in range(TG)]

            def fetch_x1(tg, ch):
                xt = xres.tile([P, 4, QG], BF16, tag=f"x{tg}_{ch}")
                nc.sync.dma_start(
                    xt[:],
                    xTr[:, ch * 4:(ch + 1) * 4, tg * QG:(tg + 1) * QG],
                )
                xtiles[tg][ch] = xt

            def fetch_x(tg):
                for ch in range(4):
                    fetch_x1(tg, ch)

            def fetch_w(lo, interleave_x=False):
                chunks = [
                    wpool.tile([P, 2, 768], BF16, tag=f"wc{wcc}", name=f"wc{wcc}")
                    for wcc in range(8)
                ]
                for wcc in range(8):
                    if interleave_x and wcc % 2 == 0:
                        fetch_x1(0, wcc // 2)
                    nc.sync.dma_start(
                        chunks[wcc][:],
                        wr[:, wcc * 2:(wcc + 1) * 2, lo:lo + 768],
                    )
                return chunks

            w_cur = fetch_w(0, interleave_x=True)

            c1 = constp.tile([P, T], BF16, tag="c1")
            c2 = constp.tile([P, T], BF16, tag="c2")
            swp = constp.tile([P, P], BF16, tag="swp")
            mk2 = constp.tile([P, 2, 2 * QG], BF16, tag="mk")
            ones_c = constp.tile([P, 1], BF16, tag="onc")
            nc.sync.dma_start(c1[:], c1d[:])
            nc.sync.dma_start(c2[:], c2d[:])
            nc.sync.dma_start(swp[:], swpd[:])
            nc.sync.dma_start(mk2[:], mkr)
            nc.sync.dma_start(ones_c[:], onesd[:])
            ebias = constp.tile([P, 1], F32, tag="ebias")
            nc.gpsimd.memset(ebias[:], EXP_BIAS)

            fetch_x(1)

            # resident y^T [4hc x 128, T] bf16
            yt_sb = [
                ytpool.tile([P, T], BF16, tag=f"yt{i}", name=f"yt{i}")
                for i in range(4)
            ]

            pending_norm = []

            def emit_norm():
                hg_, qg_, ps_y_, ps_l_ = pending_norm.pop(0)
                r_f32 = rpool.tile([1, QG], F32, tag="rf", name="r_f32")
                nc.vector.reciprocal_approx_fast(r_f32[:], ps_l_[:])
                r128 = rbigp.tile([P, QG], F32, tag="r128", name="r128")
                nc.gpsimd.partition_broadcast(r128[:], r_f32[0:1, :])
                nc.vector.tensor_mul(
                    yt_sb[hg_][:, qg_ * QG:(qg_ + 1) * QG], ps_y_[:], r128[:]
                )

            def attn_block(qh, kh, v_sb, h, hg, qg):
                # score tiles processed in PAIRS: one [128,1024] two-bank
                # psum per two key tiles -> one exp (ACT fixed overhead is
                # ~375ns/op, the attention-phase bottleneck) and one paired
                # mask mul. qk/av still per-tile on the halves.
                n_kt = 4 * qg + 4
                n_pr = n_kt // 2
                LA = 2
                ps_y = ytp.tile([P, QG], F32, tag="yt")
                l_acc = laccp.tile([P, 2 * QG], BF16, tag="l")
                p_tiles = {}

                def emit_s(pr):
                    ktA, ktB = 2 * pr, 2 * pr + 1
                    sA = ktA - 4 * qg
                    # A-half triangle shrink; B is never shrunk on qk/exp so
                    # the exp's [loA:] span reads only written psum
                    loA = sA * 128 if sA > 0 else 0
                    sB = sA + 1
                    loB = sB * 128 if sB > 0 else 0
                    ps2 = mmp.tile([P, 2 * QG], F32, tag="mm", name="ps2")
                    nc.tensor.matmul(
                        ps2[:, loA:QG],
                        r(kh[:, ktA * 128:(ktA + 1) * 128]),
                        r(qh[:, qg * QG + loA:(qg + 1) * QG]),
                        start=True,
                        stop=True,
                    )
                    nc.tensor.matmul(
                        ps2[:, QG:],
                        r(kh[:, ktB * 128:(ktB + 1) * 128]),
                        r(qh[:, qg * QG:(qg + 1) * QG]),
                        start=True,
                        stop=True,
                    )
                    p2 = pwork.tile([P, 2 * QG], BF16, tag="p", name="p2", bufs=8)
                    nc.scalar.activation(
                        p2[:, loA:], ps2[:, loA:], AF.Exp, bias=ebias[:],
                        scale=SCALE,
                    )
                    if sB >= 0:
                        dp = pr - 2 * qg  # 0 or 1: which double-mask
                        nc.vector.tensor_mul(
                            p2[:, loA:], p2[:, loA:], mk2[:, dp, loA:]
                        )
                    # softmax denominator: ONE DVE add per pair — masked
                    # junk columns of the B half are already zero, so the
                    # add can span both halves; the ones-matmul folds them
                    if pr == 0:
                        nc.vector.tensor_copy(l_acc[:], p2[:])
                    else:
                        nc.vector.tensor_add(
                            l_acc[:, loA:], l_acc[:, loA:], p2[:, loA:]
                        )
                    p_tiles[pr] = (p2, loA, loB)

                def emit_av(pr):
                    p2, loA, loB = p_tiles.pop(pr)
                    ktA, ktB = 2 * pr, 2 * pr + 1
                    nc.tensor.matmul(
                        ps_y[:, loA:],
                        r(v_sb[:, ktA, h * 128:(h + 1) * 128]),
                        r(p2[:, loA:QG]),
                        start=(ktA == 0),
                        stop=False,
                    )
                    nc.tensor.matmul(
                        ps_y[:, loB:],
                        r(v_sb[:, ktB, h * 128:(h + 1) * 128]),
                        r(p2[:, QG + loB:]),
                        start=False,
                        stop=(ktB == n_kt - 1),
                    )

                for pr in range(n_pr + LA):
                    if pr < n_pr:
                        emit_s(pr)
                    if pr == 1 and pending_norm:
                        emit_norm()
                    if pr >= LA:
                        emit_av(pr - LA)
                # softmax denominator: two accumulating ones-matmuls fold
                # the double-width l_acc halves into one [1,512] psum
                ps_l = lpp.tile([1, QG], F32, tag="l")
                nc.tensor.matmul(
                    ps_l[:], r(ones_c[:]), r(l_acc[:, 0:QG]),
                    start=True, stop=False,
                )
                nc.tensor.matmul(
                    ps_l[:], r(ones_c[:]), r(l_acc[:, QG:]),
                    start=False, stop=True,
                )
                pending_norm.append((hg, qg, ps_y, ps_l))

            def rope(j, psum, tg, q_sb, k_sb):
                dst = (q_sb if j < 2 else k_sb)[j % 2]
                dsl = dst[:, tg * QG:(tg + 1) * QG]
                qraw = work.tile([P, QG], BF16, tag="tmp", name="qraw")
                nc.scalar.activation(qraw[:], psum, AF.Copy)
                ps_sw = ytp.tile([P, QG], F32, tag="yt", name="ps_sw")
                nc.tensor.matmul(
                    ps_sw[:], swp[:], qraw[:], start=True, stop=True
                )
                t2 = work.tile([P, QG], BF16, tag="tmp", name="t2")
                c1s = c1[:, tg * QG:(tg + 1) * QG]
                c2s = c2[:, tg * QG:(tg + 1) * QG]
                nc.vector.tensor_mul(dsl, qraw[:], c1s)
                nc.vector.tensor_mul(t2[:], ps_sw[:], c2s)
                nc.vector.tensor_add(dsl, dsl, t2[:])

            def qkv_tg(w_chunks, tg, q_sb, k_sb, v_sb):
                # q/k: 4 j-tiles (q_h0, q_h1, k_h0, k_h1), N=512,
                # in two passes of 2 concurrent psums to keep mm-pool slack
                xts = xtiles[tg]

                def wsl(cc, lo, hi):
                    return w_chunks[cc // 2][:, cc % 2, lo:hi]

                for jp in range(2):
                    psq2 = mmp.tile([P, 2 * QG], F32, tag="mm", name="psq2")
                    for cc in range(16):
                        xt = xts[cc // 4][:, cc % 4, :]
                        for j in range(2):
                            nc.tensor.matmul(
                                psq2[:, j * QG:(j + 1) * QG],
                                r(wsl(cc, (jp * 2 + j) * 128, (jp * 2 + j + 1) * 128)),
                                r(xt),
                                start=(cc == 0),
                                stop=(cc == 15),
                            )
                    rope(jp * 2 + 0, psq2[:, 0:QG], tg, q_sb, k_sb)
                    rope(jp * 2 + 1, psq2[:, QG:], tg, q_sb, k_sb)
                # v: 4 t-tiles in this tg, N=256 (both heads' v); the yt
                # psum pool is idle during QKV
                for tt in range(4):
                    psv = ytp.tile([P, 256], F32, tag="yt", name="psv")
                    for cc in range(16):
                        nc.tensor.matmul(
                            psv[:],
                            r(xts[cc // 4][:, cc % 4, tt * 128:(tt + 1) * 128]),
                            r(wsl(cc, 512, 768)),
                            start=(cc == 0),
                            stop=(cc == 15),
                        )
                    nc.scalar.activation(
                        v_sb[:, tg * 4 + tt, :], psv[:], AF.Copy
                    )

            # ---- output projection: outT[oc, t] = wp^T-slice @ yT ----
            # stage copies alternate ACT/DVE so neither engine saturates
            def outproj_tg(tg):
                if tg == TG - 1:
                    # final group: single-oc stages so the very last DMA is
                    # small and issues as early as possible (shorter drain)
                    for oc in range(16):
                        ps_o = ytp.tile([P, QG], F32, tag="yt", name="ps_o")
                        for hc in range(4):
                            nc.tensor.matmul(
                                ps_o[:],
                                r(wp_sb[:, hc, oc * 128:(oc + 1) * 128]),
                                r(yt_sb[hc][:, tg * QG:(tg + 1) * QG]),
                                start=(hc == 0),
                                stop=(hc == 3),
                            )
                        stage1 = work.tile([P, QG], BF16, tag="tmp", name="stage1")
                        if oc % 2 == 0:
                            nc.scalar.activation(stage1[:], ps_o[:], AF.Copy)
                        else:
                            nc.vector.tensor_copy(stage1[:], ps_o[:])
                        nc.sync.dma_start(
                            outT[oc * 128:(oc + 1) * 128,
                                 tg * QG:(tg + 1) * QG],
                            stage1[:],
                        )
                    return
                for op_ in range(8):  # oc pairs
                    ps_o2 = mmp.tile([P, 2 * QG], F32, tag="mm", name="ps_o2")
                    for j in range(2):
                        oc = 2 * op_ + j
                        for hc in range(4):
                            nc.tensor.matmul(
                                ps_o2[:, j * QG:(j + 1) * QG],
                                r(wp_sb[:, hc, oc * 128:(oc + 1) * 128]),
                                r(yt_sb[hc][:, tg * QG:(tg + 1) * QG]),
                                start=(hc == 0),
                                stop=(hc == 3),
                            )
                    stage = work.tile([P, 2 * QG], BF16, tag="st2", name="stage", bufs=3)
                    if op_ % 2 == 0:
                        nc.scalar.activation(stage[:], ps_o2[:], AF.Copy)
                    else:
                        nc.vector.tensor_copy(stage[:], ps_o2[:])
                    for j in range(2):
                        oc = 2 * op_ + j
                        nc.sync.dma_start(
                            outT[oc * 128:(oc + 1) * 128,
                                 tg * QG:(tg + 1) * QG],
                            stage[:, j * QG:(j + 1) * QG],
                        )

            # phase-separated per pair: full QKV projection, then the
            # attention blocks (cross-engine pipelining inside each phase)
            for pair in range(2):
                w_chunks = w_cur
                if pair == 1:
                    # fetch the out-projection weights during pair1 QKV
                    wp_sb = wppool.tile([P, 4, T], BF16, tag="wp", name="wp_sb")
                    for ocq in range(4):
                        nc.sync.dma_start(
                            wp_sb[:, :, ocq * 512:(ocq + 1) * 512],
                            wpr[:, :, ocq * 512:(ocq + 1) * 512],
                        )

                q_sb = [qkres.tile([P, T], BF16, tag=f"q{h}", name=f"q{h}") for h in range(2)]
                k_sb = [qkres.tile([P, T], BF16, tag=f"k{h}", name=f"k{h}") for h in range(2)]
                v_sb = vresp.tile([P, 16, 256], BF16, tag="v")

                for tg in range(TG):
                    if pair == 0 and tg < 2:
                        fetch_x(tg + 2)
                    qkv_tg(w_chunks, tg, q_sb, k_sb, v_sb)

                if pair == 0:
                    # pair1 weights reuse pair0's chunk buffers; each DMA
                    # starts once its chunk's last pair0 read retires and
                    # streams during pair0 attention
                    w_cur = fetch_w(768)

                for h in range(2):
                    for qg in range(TG):
                        if pair == 1 and h == 1 and qg == TG - 1:
                            # fuse the first projection group in front of the
                            # last (exp-bound) attention block so its exp
                            # stream hides under projection matmuls
                            outproj_tg(0)
                        attn_block(
                            q_sb[h], k_sb[h], v_sb, h, pair * 2 + h, qg
                        )
                while pending_norm:
                    emit_norm()

            for tg in range(1, TG):
                outproj_tg(tg)

    nc.finalize()
    return nc


def _host_inputs(x, freqs_cis, w_attn, w_proj):
    """Build the 8 per-core input maps (bf16 storage)."""
    bf16 = ml_dtypes.bfloat16
    x = np.asarray(x, dtype=np.float32)
    freqs_cis = np.asarray(freqs_cis, dtype=np.float32)
    w_attn = np.asarray(w_attn, dtype=np.float32)
    w_proj = np.asarray(w_proj, dtype=np.float32)

    B = x.shape[0]
    perm = np.concatenate([np.arange(0, HSIZE, 2), np.arange(1, HSIZE, 2)])

    cos = np.ascontiguousarray(freqs_cis[:, :, 0].T)  # [64, T]
    sin = np.ascontiguousarray(freqs_cis[:, :, 1].T)
    c1 = np.concatenate([cos, cos], axis=0).astype(bf16)  # [128, T]
    c2 = np.concatenate([-sin, sin], axis=0).astype(bf16)

    kk = np.arange(P)[:, None]
    ccol = np.arange(QG)[None, :]
    msk = [(ccol >= s * 128 + kk).astype(np.float32) for s in range(4)]
    mk = np.stack(
        [np.concatenate([msk[0], msk[1]], axis=1),
         np.concatenate([msk[2], msk[3]], axis=1)], axis=0
    ).astype(bf16)  # [2,128,1024]

    swp = np.zeros((P, P), np.float32)
    for m in range(P):
        swp[(m + 64) % P, m] = 1.0
    swp = swp.astype(bf16)
    ones_col = np.ones((P, 1), bf16)

    xT = [np.ascontiguousarray(x[b].T).astype(bf16) for b in range(B)]

    in_maps = []
    for core in range(N_CORES):
        b, g = core // 4, core % 4
        blocks = []
        for pairp in range(2):
            for off in (0, C, 2 * C):  # q, k, v origins in w_attn
                for hh in range(2):
                    hglob = 4 * g + 2 * pairp + hh
                    cols = w_attn[:, off + hglob * HSIZE: off + (hglob + 1) * HSIZE]
                    if off != 2 * C:  # permute q and k, not v
                        cols = cols[:, perm]
                    blocks.append(cols)
        wcore = np.ascontiguousarray(np.concatenate(blocks, axis=1)).astype(bf16)
        wpcore = np.ascontiguousarray(w_proj[g * 512:(g + 1) * 512, :]).astype(bf16)
        in_maps.append(
            {
                "xT": xT[b],
                "w": wcore,
                "wp": wpcore,
                "c1": c1,
                "c2": c2,
                "mk": mk,
                "ones_col": ones_col,
                "swp": swp,
            }
        )
    return in_maps


_LAST_RESULT = {}


def kernel(x, freqs_cis, w_attn, w_proj):
    if _TRACE:
        _install_ntff_hook()
    in_maps = _host_inputs(x, freqs_cis, w_attn, w_proj)
    nc = build_nc()
    res = run_bass_kernel_spmd(
        nc, in_maps, core_ids=list(range(N_CORES)), trace=_TRACE
    )
    _LAST_RESULT["res"] = res

    B = x.shape[0]
    out = np.zeros((B, T, C), dtype=np.float32)
    for core in range(N_CORES):
        b = core // 4
        out[b] += np.asarray(res.results[core]["outT"], dtype=np.float32).T
    return out
